# revision 6
# baseline (speedup 1.0000x reference)
"""GCN 2-layer (PyG GCNConv x2 + ReLU) Bass kernel for Trainium2, 8-core SPMD.

Strategy:
  - Host: add self-loops, compute symmetric normalization dinv = deg^-1/2,
    fold dinv[src] into a prescaled gather table (x * dinv), shard dst nodes
    contiguously across 8 cores, sort each core's edges by dst into 128-node
    "windows", pack edges into 128-edge "chunks" (one matmul each).
    dma_gather uses int16 indices, so the node table is addressed via two
    32768-row views (LOW/HIGH); each window's edges are split into LOW chunks
    and HIGH chunks, and the kernel runs all LOW chunks (accumulating per
    window in PSUM, evicting to SBUF), then all HIGH chunks (added on top).
  - Device per core:
      Phase A (layer 1): dma_gather source rows of the prescaled x-table ->
        G [128e, d_in]; build one-hot S [128e, 128dst] on DVE (iota ==
        dst_rel); PE matmul accumulates G.T @ S into PSUM [d_in, 128dst]
        per window (aggregated x per dst, transposed).  Per window: x W1
        (PE), scale by dinv[dst], +b1, ReLU; transpose (PE); x W2; scale by
        dinv[dst]; replicate 32x -> 256B rows of the h2 table, DMA out.
      AllGather h2 shards -> full [N, 64] table.
      Phase B (layer 2): same chunk structure; gather h2 rows, matmul
        S.T @ G2[:, :2] accumulated per window; scale by dinv[dst], +b2.
"""

import numpy as np

import concourse.bass as bass
import concourse.mybir as mybir
import concourse.tile as tile
from concourse import bacc
from concourse.bass_utils import run_bass_kernel_spmd

F32 = mybir.dt.float32
BF16 = mybir.dt.bfloat16
I16 = mybir.dt.int16

N_CORES = 8
WINDOW = 128  # dst nodes per PSUM accumulation window
CHUNK = 128  # edges per matmul chunk
GSZ = 8  # max chunks per dma_gather instruction (1024 idxs, single-packet)
SBATCH = 8  # chunks per S-build DVE op
HALF = 32768  # int16 index range
REP = 64  # h2 replication (64x2 bf16 cols -> 256B rows)
GATHER_BF16 = True  # layer-1 gather table + chunk matmuls in bf16
N_QUEUES = 4  # SWDGE queues; q1-3 DGE runs async on idle Q7 core pairs


# --------------------------------------------------------------------------
# Host preprocessing
# --------------------------------------------------------------------------
def _preprocess(x, edge_index, n_cores):
    N = x.shape[0]
    src = np.concatenate(
        [np.asarray(edge_index[0], dtype=np.int64), np.arange(N, dtype=np.int64)]
    )
    dst = np.concatenate(
        [np.asarray(edge_index[1], dtype=np.int64), np.arange(N, dtype=np.int64)]
    )
    deg = np.bincount(dst, minlength=N).astype(np.float64)
    dinv = np.where(deg > 0, 1.0 / np.sqrt(deg), 0.0).astype(np.float32)

    n_local = (N + n_cores - 1) // n_cores
    w_cnt = (n_local + WINDOW - 1) // WINDOW

    order = np.argsort(dst, kind="stable")
    s_src = src[order]
    s_dst = dst[order]

    # table rows: 0 = zero, 1..N = nodes, N+1 = zero.  row(n) = n+1
    # LOW view = rows [0, min(HALF, N+2));  HIGH view = rows [HB, HB+HALF)
    HB = max(0, N + 2 - HALF)
    lowmax_row = min(HALF, N + 2)  # rows < this go to LOW chunks
    pad_low = 0  # zero row 0
    pad_high = N + 1 - HB  # zero row N+1 relative to HB

    # per (core, window): split edges into LOW (row < lowmax) and HIGH
    parts = {}  # (c, w, hi) -> (rows_arr, dstrel_arr)
    counts = np.zeros((2, n_cores, w_cnt), dtype=np.int64)
    for c in range(n_cores):
        base = c * n_local
        for w in range(w_cnt):
            wlo = base + w * WINDOW
            whi = min(base + (w + 1) * WINDOW, base + n_local, N)
            lo_i = np.searchsorted(s_dst, wlo, side="left")
            hi_i = np.searchsorted(s_dst, whi, side="left")
            rows = (s_src[lo_i:hi_i] + 1).astype(np.int64)
            rel = (s_dst[lo_i:hi_i] - wlo).astype(np.float32)
            is_lo = rows < lowmax_row
            parts[(c, w, 0)] = (rows[is_lo], rel[is_lo])
            parts[(c, w, 1)] = (rows[~is_lo] - HB, rel[~is_lo])
            counts[0, c, w] = is_lo.sum()
            counts[1, c, w] = (~is_lo).sum()

    # uniform per-window chunk counts across cores, per section
    kw_lo = np.maximum(1, np.ceil(counts[0] / CHUNK).astype(np.int64).max(axis=0))
    kw_hi = np.maximum(1, np.ceil(counts[1] / CHUNK).astype(np.int64).max(axis=0))
    T_lo, T_hi = int(kw_lo.sum()), int(kw_hi.sum())
    T = T_lo + T_hi

    # chunk order: LOW section (windows in order), then HIGH section
    chunk_win = []  # (window, first_in_sec, last_in_sec, section)
    for sec, kws in ((0, kw_lo), (1, kw_hi)):
        for w in range(w_cnt):
            for k in range(kws[w]):
                chunk_win.append((w, k == 0, k == kws[w] - 1, sec))

    per_core = []
    for c in range(n_cores):
        idx_lin = np.zeros(T * CHUNK, dtype=np.int32)
        dstrel = np.zeros((CHUNK, T), dtype=np.float32)
        t = 0
        for sec, kws, padrow in ((0, kw_lo, pad_low), (1, kw_hi, pad_high)):
            for w in range(w_cnt):
                rows, rel = parts[(c, w, sec)]
                n_e = len(rows)
                n_slots = int(kws[w]) * CHUNK
                buf = np.full(n_slots, padrow, dtype=np.int32)
                buf[:n_e] = rows
                idx_lin[t * CHUNK : t * CHUNK + n_slots] = buf
                rbuf = np.zeros(n_slots, dtype=np.float32)
                rbuf[:n_e] = rel
                dstrel[:, t : t + int(kws[w])] = rbuf.reshape(int(kws[w]), CHUNK).T
                t += int(kws[w])
        assert t == T
        # dma_gather idx layout: [128, T*8] int16; linear i = s*16 + r
        # (rows 0..15, replicated to all 128 partitions)
        idx16 = idx_lin.astype(np.int16).reshape(T * CHUNK // 16, 16).T  # [16, S]
        idx16 = np.tile(idx16, (8, 1))  # [128, S]

        dinvw = np.zeros((WINDOW, w_cnt), dtype=np.float32)
        base = c * n_local
        for w in range(w_cnt):
            wlo = base + w * WINDOW
            whi = min(wlo + WINDOW, base + n_local, N)
            if whi > wlo:
                dinvw[: whi - wlo, w] = dinv[wlo:whi]
        per_core.append({"idx16": idx16, "dstrel": dstrel, "dinvw": dinvw})

    return {
        "n_local": n_local,
        "w_cnt": w_cnt,
        "kw_lo": kw_lo,
        "kw_hi": kw_hi,
        "T_lo": T_lo,
        "T_hi": T_hi,
        "T": T,
        "HB": HB,
        "chunk_win": chunk_win,
        "dinv": dinv,
        "per_core": per_core,
    }


# --------------------------------------------------------------------------
# Device kernel builder (one program, SPMD across cores)
# --------------------------------------------------------------------------
def _build(nc, *, N, n_local, d_in, d_hid, n_cls, pp, n_cores, dt_gat):
    Relu = mybir.ActivationFunctionType.Relu
    Copy = mybir.ActivationFunctionType.Copy
    T, T_lo = pp["T"], pp["T_lo"]
    w_cnt, HB = pp["w_cnt"], pp["HB"]
    chunk_win = pp["chunk_win"]
    d_rep = REP * n_cls  # 64 cols of f32 -> 256B rows

    xtab = nc.dram_tensor("xtab", [N + 2, d_in], dt_gat, kind="ExternalInput")
    w1 = nc.dram_tensor("w1", [d_in, d_hid], F32, kind="ExternalInput")
    w2 = nc.dram_tensor("w2", [d_hid, n_cls], F32, kind="ExternalInput")
    b1bc = nc.dram_tensor("b1bc", [WINDOW, d_hid], F32, kind="ExternalInput")
    b2bc = nc.dram_tensor("b2bc", [WINDOW, n_cls], F32, kind="ExternalInput")
    iota = nc.dram_tensor("iota", [CHUNK, SBATCH * WINDOW], F32, kind="ExternalInput")
    ident = nc.dram_tensor("ident", [WINDOW, WINDOW], F32, kind="ExternalInput")
    idx_t = nc.dram_tensor("idx16", [CHUNK, T * 8], I16, kind="ExternalInput")
    dstrel_t = nc.dram_tensor("dstrel", [CHUNK, T], F32, kind="ExternalInput")
    dinvw_t = nc.dram_tensor("dinvw", [WINDOW, w_cnt], F32, kind="ExternalInput")
    out_t = nc.dram_tensor("out", [n_local, n_cls], F32, kind="ExternalOutput")

    h2loc = nc.dram_tensor("h2loc", [n_local, d_rep], BF16)
    h2tab = nc.dram_tensor("h2tab", [N + 2, d_rep], BF16, addr_space="Shared")

    # per-section gather groups: (sec, t0, n)
    groups = []
    for sec, tlo, thi in ((0, 0, T_lo), (1, T_lo, T)):
        t0 = tlo
        while t0 < thi:
            n = min(GSZ, thi - t0)
            groups.append((sec, t0, n))
            t0 += n

    def tab_view(tab):
        return [
            tab[0 : min(HALF, N + 2), :],
            tab[HB : min(HB + HALF, N + 2), :],
        ]

    with tile.TileContext(nc) as tc:
        with (
            tc.tile_pool(name="const", bufs=1) as cpool,
            tc.tile_pool(name="gbuf", bufs=10) as gpool,
            tc.tile_pool(name="g2buf", bufs=10) as g2pool,
            tc.tile_pool(name="sbat", bufs=4) as spool,
            tc.tile_pool(name="sbat2", bufs=4) as s2pool,
            tc.tile_pool(name="wtmp", bufs=8) as wpool,
            tc.tile_pool(name="aggs", bufs=1) as apool,
            tc.tile_pool(name="psA", bufs=5, space="PSUM") as psA,
            tc.tile_pool(name="psW", bufs=3, space="PSUM") as psW,
        ):
            # ---- constants into SBUF ----
            w1_sb = cpool.tile([d_in, d_hid], F32, tag="w1")
            nc.sync.dma_start(out=w1_sb[:], in_=w1[:])
            w2_sb = cpool.tile([d_hid, n_cls], F32, tag="w2")
            nc.sync.dma_start(out=w2_sb[:], in_=w2[:])
            b1_sb = cpool.tile([WINDOW, d_hid], F32, tag="b1")
            nc.sync.dma_start(out=b1_sb[:], in_=b1bc[:])
            b2_sb = cpool.tile([WINDOW, n_cls], F32, tag="b2")
            nc.sync.dma_start(out=b2_sb[:], in_=b2bc[:])
            iota_sb = cpool.tile([CHUNK, SBATCH * WINDOW], F32, tag="iota")
            nc.sync.dma_start(out=iota_sb[:], in_=iota[:])
            id_sb = cpool.tile([WINDOW, WINDOW], F32, tag="ident")
            nc.sync.dma_start(out=id_sb[:], in_=ident[:])
            idx_sb = cpool.tile([CHUNK, T * 8], I16, tag="idx")
            nc.sync.dma_start(out=idx_sb[:], in_=idx_t[:])
            dstrel_sb = cpool.tile([CHUNK, T], F32, tag="dstrel")
            nc.sync.dma_start(out=dstrel_sb[:], in_=dstrel_t[:])
            dinvw_sb = cpool.tile([WINDOW, w_cnt], F32, tag="dinvw")
            nc.sync.dma_start(out=dinvw_sb[:], in_=dinvw_t[:])

            zrow = cpool.tile([1, d_rep], BF16, tag="zrow")
            nc.vector.memset(zrow[:], 0.0)
            nc.sync.dma_start(out=h2tab[0:1, :], in_=zrow[:1, :])
            nc.sync.dma_start(out=h2tab[N + 1 : N + 2, :], in_=zrow[:1, :])

            def build_s(pool, t0, n, nm):
                """one-hot S for chunks [t0, t0+n) in one DVE op."""
                s_tile = pool.tile([CHUNK, SBATCH * WINDOW], BF16, tag="s", name=nm)
                rel_b = (
                    dstrel_sb[:, t0 : t0 + n]
                    .rearrange("p (b one) -> p b one", one=1)
                    .to_broadcast([CHUNK, n, WINDOW])
                )
                io_v = iota_sb[:, : n * WINDOW].rearrange("p (b j) -> p b j", j=WINDOW)
                s_v = s_tile[:, : n * WINDOW].rearrange("p (b j) -> p b j", j=WINDOW)
                nc.vector.tensor_tensor(
                    out=s_v, in0=io_v, in1=rel_b, op=mybir.AluOpType.is_equal
                )
                return s_tile

            # per-window accumulators in SBUF (LOW evicts, HIGH adds on top)
            aggT_sb = apool.tile([d_in, w_cnt * WINDOW], F32, tag="aggT")
            out2_sb = apool.tile([WINDOW, w_cnt * n_cls], F32, tag="out2")

            # =========================== PHASE A ===========================
            psum_of_win = {}
            for gi, (sec, t0, n) in enumerate(groups):
                gb = gpool.tile([CHUNK, GSZ, d_in], dt_gat, tag="g", name="gb")
                nc.gpsimd.dma_gather(
                    gb[:, :n, :],
                    tab_view(xtab)[sec],
                    idx_sb[:, t0 * 8 : (t0 + n) * 8],
                    n * CHUNK,
                    n * CHUNK,
                    d_in,
                    single_packet=True,
                    queue_num=gi % N_QUEUES,
                )
                for bt0 in range(t0, t0 + n, SBATCH):
                    bn = min(SBATCH, t0 + n - bt0)
                    s_tile = build_s(spool, bt0, bn, "sA")
                    for t in range(bt0, bt0 + bn):
                        j = t - bt0
                        w, first, last, _sec = chunk_win[t]
                        if first:
                            psum_of_win[w] = psA.tile(
                                [d_in, WINDOW], F32, tag="agg", name="aggps"
                            )
                        nc.tensor.matmul(
                            out=psum_of_win[w][:],
                            lhsT=gb[:, t - t0, :],
                            rhs=s_tile[:, j * WINDOW : (j + 1) * WINDOW],
                            start=first,
                            stop=last,
                        )
                        if not last:
                            continue
                        ps = psum_of_win.pop(w)
                        wsl = aggT_sb[:, w * WINDOW : (w + 1) * WINDOW]
                        if _sec == 0:
                            nc.scalar.activation(out=wsl, in_=ps[:], func=Copy)
                        else:
                            nc.vector.tensor_tensor(
                                out=wsl, in0=ps[:], in1=wsl, op=mybir.AluOpType.add
                            )
                            _window_epilogue_A(
                                nc, w, wsl, wpool, psW, w1_sb, w2_sb, b1_sb,
                                dinvw_sb, id_sb, h2loc, n_local, d_in, d_hid,
                                n_cls, d_rep,
                            )

            # ======================= h2 exchange ==========================
            if n_cores > 1:
                nc.gpsimd.collective_compute(
                    "AllGather",
                    mybir.AluOpType.bypass,
                    replica_groups=[list(range(n_cores))],
                    ins=[h2loc[:]],
                    outs=[h2tab[1 : 1 + n_cores * n_local, :]],
                )
            else:
                nc.sync.dma_start(out=h2tab[1 : 1 + n_local, :], in_=h2loc[:])

            # =========================== PHASE B ===========================
            psum_of_win = {}
            for gi, (sec, t0, n) in enumerate(groups):
                g2 = g2pool.tile([CHUNK, GSZ, d_rep], BF16, tag="g2", name="g2b")
                nc.gpsimd.dma_gather(
                    g2[:, :n, :],
                    tab_view(h2tab)[sec],
                    idx_sb[:, t0 * 8 : (t0 + n) * 8],
                    n * CHUNK,
                    n * CHUNK,
                    d_rep,
                    single_packet=True,
                    queue_num=gi % N_QUEUES,
                )
                for bt0 in range(t0, t0 + n, SBATCH):
                    bn = min(SBATCH, t0 + n - bt0)
                    s_tile = build_s(s2pool, bt0, bn, "sB")
                    for t in range(bt0, bt0 + bn):
                        j = t - bt0
                        w, first, last, _sec = chunk_win[t]
                        if first:
                            psum_of_win[w] = psA.tile(
                                [WINDOW, n_cls], F32, tag="agg", name="agg2ps"
                            )
                        nc.tensor.matmul(
                            out=psum_of_win[w][:],
                            lhsT=s_tile[:, j * WINDOW : (j + 1) * WINDOW],
                            rhs=g2[:, t - t0, :n_cls],
                            start=first,
                            stop=last,
                        )
                        if not last:
                            continue
                        ps = psum_of_win.pop(w)
                        osl = out2_sb[:, w * n_cls : (w + 1) * n_cls]
                        if _sec == 0:
                            nc.scalar.activation(out=osl, in_=ps[:], func=Copy)
                        else:
                            ob = wpool.tile([WINDOW, n_cls], F32, tag="ob")
                            nc.vector.tensor_tensor(
                                out=ob[:], in0=ps[:], in1=osl, op=mybir.AluOpType.add
                            )
                            ob2 = wpool.tile([WINDOW, n_cls], F32, tag="ob2")
                            nc.vector.tensor_scalar(
                                out=ob2[:],
                                in0=ob[:],
                                scalar1=dinvw_sb[:, w : w + 1],
                                scalar2=None,
                                op0=mybir.AluOpType.mult,
                            )
                            ob3 = wpool.tile([WINDOW, n_cls], F32, tag="ob3")
                            nc.vector.tensor_tensor(
                                out=ob3[:], in0=ob2[:], in1=b2_sb[:],
                                op=mybir.AluOpType.add,
                            )
                            nrows = min(WINDOW, n_local - w * WINDOW)
                            nc.sync.dma_start(
                                out=out_t[w * WINDOW : w * WINDOW + nrows, :],
                                in_=ob3[:nrows, :],
                            )

    nc.compile()
    return nc


def _window_epilogue_A(
    nc, w, aggT, wpool, psW, w1_sb, w2_sb, b1_sb, dinvw_sb, id_sb,
    h2loc, n_local, d_in, d_hid, n_cls, d_rep,
):
    """aggT [d_in, WINDOW] in SBUF -> replicated h2 rows in DRAM."""
    Relu = mybir.ActivationFunctionType.Relu
    Copy = mybir.ActivationFunctionType.Copy

    # h1 [dst, hid] = aggT.T @ W1
    h1_ps = psW.tile([WINDOW, d_hid], F32, tag="wps", name="h1_ps")
    nc.tensor.matmul(out=h1_ps[:], lhsT=aggT, rhs=w1_sb[:], start=True, stop=True)
    # scale by dinv[dst] (per-partition), + b1, relu
    r_sb = wpool.tile([WINDOW, d_hid], F32, tag="r")
    nc.vector.tensor_scalar(
        out=r_sb[:],
        in0=h1_ps[:],
        scalar1=dinvw_sb[:, w : w + 1],
        scalar2=None,
        op0=mybir.AluOpType.mult,
    )
    r2_sb = wpool.tile([WINDOW, d_hid], F32, tag="r2")
    nc.vector.tensor_tensor(
        out=r2_sb[:], in0=r_sb[:], in1=b1_sb[:], op=mybir.AluOpType.add
    )
    r3_sb = wpool.tile([WINDOW, d_hid], F32, tag="r3")
    nc.scalar.activation(out=r3_sb[:], in_=r2_sb[:], func=Relu)
    # transpose -> [hid, dst]
    rT_ps = psW.tile([d_hid, WINDOW], F32, tag="wps", name="rT_ps")
    nc.tensor.transpose(out=rT_ps[:], in_=r3_sb[:], identity=id_sb[:])
    rT_sb = wpool.tile([d_hid, WINDOW], F32, tag="rTs")
    nc.scalar.activation(out=rT_sb[:], in_=rT_ps[:], func=Copy)
    # h2 [dst, n_cls] = rT.T @ W2; scale by dinv[dst]; replicate REP x
    h2_ps = psW.tile([WINDOW, n_cls], F32, tag="wps", name="h2_ps")
    nc.tensor.matmul(out=h2_ps[:], lhsT=rT_sb[:], rhs=w2_sb[:], start=True, stop=True)
    h2_sb = wpool.tile([WINDOW, d_rep], BF16, tag="h2s")
    nc.vector.tensor_scalar(
        out=h2_sb[:].rearrange("p (r c) -> p r c", c=n_cls),
        in0=h2_ps[:]
        .rearrange("p (one c) -> p one c", one=1)
        .to_broadcast([WINDOW, REP, n_cls]),
        scalar1=dinvw_sb[:, w : w + 1],
        scalar2=None,
        op0=mybir.AluOpType.mult,
    )
    nrows = min(WINDOW, n_local - w * WINDOW)
    nc.sync.dma_start(
        out=h2loc[w * WINDOW : w * WINDOW + nrows, :], in_=h2_sb[:nrows, :]
    )


# --------------------------------------------------------------------------
# Entry point
# --------------------------------------------------------------------------
def _make_inputs(x, W1, b1, W2, b2, pp, dt_np):
    N, d_in = x.shape
    W1 = np.asarray(W1, np.float32)
    b1 = np.asarray(b1, np.float32)
    W2 = np.asarray(W2, np.float32)
    b2 = np.asarray(b2, np.float32)
    d_hid = W1.shape[1]
    n_cls = W2.shape[1]
    xtab = np.concatenate(
        [
            np.zeros((1, d_in), np.float32),
            x * pp["dinv"][:, None],
            np.zeros((1, d_in), np.float32),
        ]
    ).astype(dt_np)
    iota_arr = np.broadcast_to(
        np.tile(np.arange(WINDOW, dtype=np.float32), SBATCH),
        (CHUNK, SBATCH * WINDOW),
    ).copy()
    shared = {
        "xtab": xtab,
        "w1": W1,
        "w2": W2,
        "b1bc": np.broadcast_to(b1, (WINDOW, d_hid)).astype(np.float32).copy(),
        "b2bc": np.broadcast_to(b2, (WINDOW, n_cls)).astype(np.float32).copy(),
        "iota": iota_arr,
        "ident": np.eye(WINDOW, dtype=np.float32),
    }
    in_maps = []
    for pc in pp["per_core"]:
        m = dict(shared)
        m["idx16"] = pc["idx16"]
        m["dstrel"] = pc["dstrel"]
        m["dinvw"] = pc["dinvw"]
        in_maps.append(m)
    return in_maps


def _run(x, edge_index, W1, b1, W2, b2, n_cores, trace=False):
    x = np.asarray(x, dtype=np.float32)
    N, d_in = x.shape
    d_hid = np.asarray(W1).shape[1]
    n_cls = np.asarray(W2).shape[1]
    assert d_in == 128 and d_hid == 128

    pp = _preprocess(x, edge_index, n_cores)
    dt_gat = BF16 if GATHER_BF16 else F32
    np_gat = np.dtype("bfloat16") if GATHER_BF16 else np.dtype("float32")

    nc = bacc.Bacc("TRN2", target_bir_lowering=False, debug=False,
                   num_swdge_queues=N_QUEUES,
                   dynamic_dma_scratch_size=65536)
    _build(
        nc,
        N=N,
        n_local=pp["n_local"],
        d_in=d_in,
        d_hid=d_hid,
        n_cls=n_cls,
        pp=pp,
        n_cores=n_cores,
        dt_gat=dt_gat,
    )

    import ml_dtypes  # noqa

    in_maps = _make_inputs(x, W1, b1, W2, b2, pp, np_gat)
    res = run_bass_kernel_spmd(nc, in_maps, list(range(n_cores)), trace=trace)
    outs = [res.results[c]["out"] for c in range(n_cores)]
    full = np.concatenate(outs, axis=0)[:N]
    return full.astype(np.float32), res


def kernel(x, edge_index, W1, b1, W2, b2):
    out, _ = _run(x, edge_index, W1, b1, W2, b2, N_CORES)
    return out



# revision 7
# speedup vs baseline: 1.0234x; 1.0234x over previous
"""GCN 2-layer (PyG GCNConv x2 + ReLU) Bass kernel for Trainium2, 8-core SPMD.

Strategy:
  - Host: add self-loops, compute symmetric normalization dinv = deg^-1/2,
    fold dinv[src] into a prescaled gather table (x * dinv), shard dst nodes
    contiguously across 8 cores, sort each core's edges by dst into 128-node
    "windows", pack edges into 128-edge "chunks" (one matmul each).
    dma_gather uses int16 indices, so the node table is addressed via two
    32768-row views (LOW/HIGH); each window's edges are split into LOW chunks
    and HIGH chunks, and the kernel runs all LOW chunks (accumulating per
    window in PSUM, evicting to SBUF), then all HIGH chunks (added on top).
  - Device per core:
      Phase A (layer 1): dma_gather source rows of the prescaled x-table ->
        G [128e, d_in]; build one-hot S [128e, 128dst] on DVE (iota ==
        dst_rel); PE matmul accumulates G.T @ S into PSUM [d_in, 128dst]
        per window (aggregated x per dst, transposed).  Per window: x W1
        (PE), scale by dinv[dst], +b1, ReLU; transpose (PE); x W2; scale by
        dinv[dst]; replicate 32x -> 256B rows of the h2 table, DMA out.
      AllGather h2 shards -> full [N, 64] table.
      Phase B (layer 2): same chunk structure; gather h2 rows, matmul
        S.T @ G2[:, :2] accumulated per window; scale by dinv[dst], +b2.
"""

import numpy as np

import concourse.bass as bass
import concourse.mybir as mybir
import concourse.tile as tile
from concourse import bacc
from concourse.bass_utils import run_bass_kernel_spmd

F32 = mybir.dt.float32
BF16 = mybir.dt.bfloat16
I16 = mybir.dt.int16

N_CORES = 8
WINDOW = 128  # dst nodes per PSUM accumulation window
CHUNK = 128  # edges per matmul chunk
GSZ = 8  # max chunks per dma_gather instruction (1024 idxs, single-packet)
SBATCH = 8  # chunks per S-build DVE op
HALF = 32768  # int16 index range
REP = 64  # h2 replication (64x2 bf16 cols -> 256B rows)
GATHER_BF16 = True  # layer-1 gather table + chunk matmuls in bf16
N_QUEUES = 4  # SWDGE queues; q1-3 DGE runs async on idle Q7 core pairs


# --------------------------------------------------------------------------
# Host preprocessing
# --------------------------------------------------------------------------
def _preprocess(x, edge_index, n_cores):
    N = x.shape[0]
    src = np.concatenate(
        [np.asarray(edge_index[0], dtype=np.int64), np.arange(N, dtype=np.int64)]
    )
    dst = np.concatenate(
        [np.asarray(edge_index[1], dtype=np.int64), np.arange(N, dtype=np.int64)]
    )
    deg = np.bincount(dst, minlength=N).astype(np.float64)
    dinv = np.where(deg > 0, 1.0 / np.sqrt(deg), 0.0).astype(np.float32)

    n_local = (N + n_cores - 1) // n_cores
    w_cnt = (n_local + WINDOW - 1) // WINDOW

    order = np.argsort(dst, kind="stable")
    s_src = src[order]
    s_dst = dst[order]

    # table rows: 0 = zero, 1..N = nodes, N+1 = zero.  row(n) = n+1
    # LOW view = rows [0, min(HALF, N+2));  HIGH view = rows [HB, HB+HALF)
    HB = max(0, N + 2 - HALF)
    lowmax_row = min(HALF, N + 2)  # rows < this go to LOW chunks
    pad_low = 0  # zero row 0
    pad_high = N + 1 - HB  # zero row N+1 relative to HB

    # per (core, window): split edges into LOW (row < lowmax) and HIGH
    parts = {}  # (c, w, hi) -> (rows_arr, dstrel_arr)
    counts = np.zeros((2, n_cores, w_cnt), dtype=np.int64)
    for c in range(n_cores):
        base = c * n_local
        for w in range(w_cnt):
            wlo = base + w * WINDOW
            whi = min(base + (w + 1) * WINDOW, base + n_local, N)
            lo_i = np.searchsorted(s_dst, wlo, side="left")
            hi_i = np.searchsorted(s_dst, whi, side="left")
            rows = (s_src[lo_i:hi_i] + 1).astype(np.int64)
            rel = (s_dst[lo_i:hi_i] - wlo).astype(np.float32)
            is_lo = rows < lowmax_row
            parts[(c, w, 0)] = (rows[is_lo], rel[is_lo])
            parts[(c, w, 1)] = (rows[~is_lo] - HB, rel[~is_lo])
            counts[0, c, w] = is_lo.sum()
            counts[1, c, w] = (~is_lo).sum()

    # uniform per-window chunk counts across cores, per section
    kw_lo = np.maximum(1, np.ceil(counts[0] / CHUNK).astype(np.int64).max(axis=0))
    kw_hi = np.maximum(1, np.ceil(counts[1] / CHUNK).astype(np.int64).max(axis=0))
    T_lo, T_hi = int(kw_lo.sum()), int(kw_hi.sum())
    T = T_lo + T_hi

    # chunk order: LOW section (windows in order), then HIGH section
    chunk_win = []  # (window, first_in_sec, last_in_sec, section)
    for sec, kws in ((0, kw_lo), (1, kw_hi)):
        for w in range(w_cnt):
            for k in range(kws[w]):
                chunk_win.append((w, k == 0, k == kws[w] - 1, sec))

    per_core = []
    for c in range(n_cores):
        idx_lin = np.zeros(T * CHUNK, dtype=np.int32)
        dstrel = np.zeros((CHUNK, T), dtype=np.float32)
        t = 0
        for sec, kws, padrow in ((0, kw_lo, pad_low), (1, kw_hi, pad_high)):
            for w in range(w_cnt):
                rows, rel = parts[(c, w, sec)]
                n_e = len(rows)
                n_slots = int(kws[w]) * CHUNK
                buf = np.full(n_slots, padrow, dtype=np.int32)
                buf[:n_e] = rows
                idx_lin[t * CHUNK : t * CHUNK + n_slots] = buf
                rbuf = np.zeros(n_slots, dtype=np.float32)
                rbuf[:n_e] = rel
                dstrel[:, t : t + int(kws[w])] = rbuf.reshape(int(kws[w]), CHUNK).T
                t += int(kws[w])
        assert t == T
        # dma_gather idx layout: [128, T*8] int16; linear i = s*16 + r
        # (rows 0..15, replicated to all 128 partitions)
        idx16 = idx_lin.astype(np.int16).reshape(T * CHUNK // 16, 16).T  # [16, S]
        idx16 = np.tile(idx16, (8, 1))  # [128, S]

        dinvw = np.zeros((WINDOW, w_cnt), dtype=np.float32)
        base = c * n_local
        for w in range(w_cnt):
            wlo = base + w * WINDOW
            whi = min(wlo + WINDOW, base + n_local, N)
            if whi > wlo:
                dinvw[: whi - wlo, w] = dinv[wlo:whi]
        per_core.append({"idx16": idx16, "dstrel": dstrel, "dinvw": dinvw})

    return {
        "n_local": n_local,
        "w_cnt": w_cnt,
        "kw_lo": kw_lo,
        "kw_hi": kw_hi,
        "T_lo": T_lo,
        "T_hi": T_hi,
        "T": T,
        "HB": HB,
        "chunk_win": chunk_win,
        "dinv": dinv,
        "per_core": per_core,
    }


# --------------------------------------------------------------------------
# Device kernel builder (one program, SPMD across cores)
# --------------------------------------------------------------------------
def _build(nc, *, N, n_local, d_in, d_hid, n_cls, pp, n_cores, dt_gat):
    Relu = mybir.ActivationFunctionType.Relu
    Copy = mybir.ActivationFunctionType.Copy
    T, T_lo = pp["T"], pp["T_lo"]
    w_cnt, HB = pp["w_cnt"], pp["HB"]
    chunk_win = pp["chunk_win"]
    d_rep = REP * n_cls  # 64 cols of f32 -> 256B rows

    xtab = nc.dram_tensor("xtab", [N + 2, d_in], dt_gat, kind="ExternalInput")
    w1 = nc.dram_tensor("w1", [d_in, d_hid], F32, kind="ExternalInput")
    w2 = nc.dram_tensor("w2", [d_hid, n_cls], F32, kind="ExternalInput")
    b1bc = nc.dram_tensor("b1bc", [WINDOW, d_hid], F32, kind="ExternalInput")
    b2bc = nc.dram_tensor("b2bc", [WINDOW, n_cls], F32, kind="ExternalInput")
    iota = nc.dram_tensor("iota", [CHUNK, SBATCH * WINDOW], F32, kind="ExternalInput")
    ident = nc.dram_tensor("ident", [WINDOW, WINDOW], F32, kind="ExternalInput")
    idx_t = nc.dram_tensor("idx16", [CHUNK, T * 8], I16, kind="ExternalInput")
    dstrel_t = nc.dram_tensor("dstrel", [CHUNK, T], F32, kind="ExternalInput")
    dinvw_t = nc.dram_tensor("dinvw", [WINDOW, w_cnt], F32, kind="ExternalInput")
    out_t = nc.dram_tensor("out", [n_local, n_cls], F32, kind="ExternalOutput")

    h2loc = nc.dram_tensor("h2loc", [n_local, d_rep], BF16)
    h2tab = nc.dram_tensor("h2tab", [N + 2, d_rep], BF16, addr_space="Shared")

    # per-section gather groups: (sec, t0, n)
    groups = []
    for sec, tlo, thi in ((0, 0, T_lo), (1, T_lo, T)):
        t0 = tlo
        while t0 < thi:
            n = min(GSZ, thi - t0)
            groups.append((sec, t0, n))
            t0 += n

    def tab_view(tab):
        return [
            tab[0 : min(HALF, N + 2), :],
            tab[HB : min(HB + HALF, N + 2), :],
        ]

    with tile.TileContext(nc) as tc:
        with (
            tc.tile_pool(name="const", bufs=1) as cpool,
            tc.tile_pool(name="gbuf", bufs=10) as gpool,
            tc.tile_pool(name="g2buf", bufs=10) as g2pool,
            tc.tile_pool(name="sbat", bufs=4) as spool,
            tc.tile_pool(name="sbat2", bufs=4) as s2pool,
            tc.tile_pool(name="wtmp", bufs=8) as wpool,
            tc.tile_pool(name="aggs", bufs=1) as apool,
            tc.tile_pool(name="psA", bufs=5, space="PSUM") as psA,
            tc.tile_pool(name="psW", bufs=3, space="PSUM") as psW,
        ):
            # ---- constants into SBUF ----
            w1_sb = cpool.tile([d_in, d_hid], F32, tag="w1")
            nc.sync.dma_start(out=w1_sb[:], in_=w1[:])
            w2_sb = cpool.tile([d_hid, n_cls], F32, tag="w2")
            nc.sync.dma_start(out=w2_sb[:], in_=w2[:])
            b1_sb = cpool.tile([WINDOW, d_hid], F32, tag="b1")
            nc.sync.dma_start(out=b1_sb[:], in_=b1bc[:])
            b2_sb = cpool.tile([WINDOW, n_cls], F32, tag="b2")
            nc.sync.dma_start(out=b2_sb[:], in_=b2bc[:])
            iota_sb = cpool.tile([CHUNK, SBATCH * WINDOW], F32, tag="iota")
            nc.sync.dma_start(out=iota_sb[:], in_=iota[:])
            id_sb = cpool.tile([WINDOW, WINDOW], F32, tag="ident")
            nc.sync.dma_start(out=id_sb[:], in_=ident[:])
            idx_sb = cpool.tile([CHUNK, T * 8], I16, tag="idx")
            nc.sync.dma_start(out=idx_sb[:], in_=idx_t[:])
            dstrel_sb = cpool.tile([CHUNK, T], F32, tag="dstrel")
            nc.sync.dma_start(out=dstrel_sb[:], in_=dstrel_t[:])
            dinvw_sb = cpool.tile([WINDOW, w_cnt], F32, tag="dinvw")
            nc.sync.dma_start(out=dinvw_sb[:], in_=dinvw_t[:])

            zrow = cpool.tile([1, d_rep], BF16, tag="zrow")
            nc.vector.memset(zrow[:], 0.0)
            nc.sync.dma_start(out=h2tab[0:1, :], in_=zrow[:1, :])
            nc.sync.dma_start(out=h2tab[N + 1 : N + 2, :], in_=zrow[:1, :])

            def build_s(pool, t0, n, nm):
                """one-hot S for chunks [t0, t0+n) in one DVE op."""
                s_tile = pool.tile([CHUNK, SBATCH * WINDOW], BF16, tag="s", name=nm)
                rel_b = (
                    dstrel_sb[:, t0 : t0 + n]
                    .rearrange("p (b one) -> p b one", one=1)
                    .to_broadcast([CHUNK, n, WINDOW])
                )
                io_v = iota_sb[:, : n * WINDOW].rearrange("p (b j) -> p b j", j=WINDOW)
                s_v = s_tile[:, : n * WINDOW].rearrange("p (b j) -> p b j", j=WINDOW)
                nc.vector.tensor_tensor(
                    out=s_v, in0=io_v, in1=rel_b, op=mybir.AluOpType.is_equal
                )
                return s_tile

            # per-window accumulators in SBUF (LOW evicts, HIGH adds on top)
            aggT_sb = apool.tile([d_in, w_cnt * WINDOW], F32, tag="aggT")
            out2_sb = apool.tile([WINDOW, w_cnt * n_cls], F32, tag="out2")

            # =========================== PHASE A ===========================
            psum_of_win = {}
            for gi, (sec, t0, n) in enumerate(groups):
                gb = gpool.tile([CHUNK, GSZ, d_in], dt_gat, tag="g", name="gb")
                nc.gpsimd.dma_gather(
                    gb[:, :n, :],
                    tab_view(xtab)[sec],
                    idx_sb[:, t0 * 8 : (t0 + n) * 8],
                    n * CHUNK,
                    n * CHUNK,
                    d_in,
                    single_packet=True,
                    queue_num=1 + gi % 3,
                )
                for bt0 in range(t0, t0 + n, SBATCH):
                    bn = min(SBATCH, t0 + n - bt0)
                    s_tile = build_s(spool, bt0, bn, "sA")
                    for t in range(bt0, bt0 + bn):
                        j = t - bt0
                        w, first, last, _sec = chunk_win[t]
                        if first:
                            psum_of_win[w] = psA.tile(
                                [d_in, WINDOW], F32, tag="agg", name="aggps"
                            )
                        nc.tensor.matmul(
                            out=psum_of_win[w][:],
                            lhsT=gb[:, t - t0, :],
                            rhs=s_tile[:, j * WINDOW : (j + 1) * WINDOW],
                            start=first,
                            stop=last,
                        )
                        if not last:
                            continue
                        ps = psum_of_win.pop(w)
                        wsl = aggT_sb[:, w * WINDOW : (w + 1) * WINDOW]
                        if _sec == 0:
                            nc.scalar.activation(out=wsl, in_=ps[:], func=Copy)
                        else:
                            nc.vector.tensor_tensor(
                                out=wsl, in0=ps[:], in1=wsl, op=mybir.AluOpType.add
                            )
                            _window_epilogue_A(
                                nc, w, wsl, wpool, psW, w1_sb, w2_sb, b1_sb,
                                dinvw_sb, id_sb, h2loc, n_local, d_in, d_hid,
                                n_cls, d_rep,
                            )

            # ======================= h2 exchange ==========================
            if n_cores > 1:
                nc.gpsimd.collective_compute(
                    "AllGather",
                    mybir.AluOpType.bypass,
                    replica_groups=[list(range(n_cores))],
                    ins=[h2loc[:]],
                    outs=[h2tab[1 : 1 + n_cores * n_local, :]],
                )
            else:
                nc.sync.dma_start(out=h2tab[1 : 1 + n_local, :], in_=h2loc[:])

            # =========================== PHASE B ===========================
            psum_of_win = {}
            for gi, (sec, t0, n) in enumerate(groups):
                g2 = g2pool.tile([CHUNK, GSZ, d_rep], BF16, tag="g2", name="g2b")
                nc.gpsimd.dma_gather(
                    g2[:, :n, :],
                    tab_view(h2tab)[sec],
                    idx_sb[:, t0 * 8 : (t0 + n) * 8],
                    n * CHUNK,
                    n * CHUNK,
                    d_rep,
                    single_packet=True,
                    queue_num=1 + gi % 3,
                )
                for bt0 in range(t0, t0 + n, SBATCH):
                    bn = min(SBATCH, t0 + n - bt0)
                    s_tile = build_s(s2pool, bt0, bn, "sB")
                    for t in range(bt0, bt0 + bn):
                        j = t - bt0
                        w, first, last, _sec = chunk_win[t]
                        if first:
                            psum_of_win[w] = psA.tile(
                                [WINDOW, n_cls], F32, tag="agg", name="agg2ps"
                            )
                        nc.tensor.matmul(
                            out=psum_of_win[w][:],
                            lhsT=s_tile[:, j * WINDOW : (j + 1) * WINDOW],
                            rhs=g2[:, t - t0, :n_cls],
                            start=first,
                            stop=last,
                        )
                        if not last:
                            continue
                        ps = psum_of_win.pop(w)
                        osl = out2_sb[:, w * n_cls : (w + 1) * n_cls]
                        if _sec == 0:
                            nc.scalar.activation(out=osl, in_=ps[:], func=Copy)
                        else:
                            ob = wpool.tile([WINDOW, n_cls], F32, tag="ob")
                            nc.vector.tensor_tensor(
                                out=ob[:], in0=ps[:], in1=osl, op=mybir.AluOpType.add
                            )
                            ob2 = wpool.tile([WINDOW, n_cls], F32, tag="ob2")
                            nc.vector.tensor_scalar(
                                out=ob2[:],
                                in0=ob[:],
                                scalar1=dinvw_sb[:, w : w + 1],
                                scalar2=None,
                                op0=mybir.AluOpType.mult,
                            )
                            ob3 = wpool.tile([WINDOW, n_cls], F32, tag="ob3")
                            nc.vector.tensor_tensor(
                                out=ob3[:], in0=ob2[:], in1=b2_sb[:],
                                op=mybir.AluOpType.add,
                            )
                            nrows = min(WINDOW, n_local - w * WINDOW)
                            nc.sync.dma_start(
                                out=out_t[w * WINDOW : w * WINDOW + nrows, :],
                                in_=ob3[:nrows, :],
                            )

    nc.compile()
    return nc


def _window_epilogue_A(
    nc, w, aggT, wpool, psW, w1_sb, w2_sb, b1_sb, dinvw_sb, id_sb,
    h2loc, n_local, d_in, d_hid, n_cls, d_rep,
):
    """aggT [d_in, WINDOW] in SBUF -> replicated h2 rows in DRAM."""
    Relu = mybir.ActivationFunctionType.Relu
    Copy = mybir.ActivationFunctionType.Copy

    # h1 [dst, hid] = aggT.T @ W1
    h1_ps = psW.tile([WINDOW, d_hid], F32, tag="wps", name="h1_ps")
    nc.tensor.matmul(out=h1_ps[:], lhsT=aggT, rhs=w1_sb[:], start=True, stop=True)
    # scale by dinv[dst] (per-partition), + b1, relu
    r_sb = wpool.tile([WINDOW, d_hid], F32, tag="r")
    nc.vector.tensor_scalar(
        out=r_sb[:],
        in0=h1_ps[:],
        scalar1=dinvw_sb[:, w : w + 1],
        scalar2=None,
        op0=mybir.AluOpType.mult,
    )
    r2_sb = wpool.tile([WINDOW, d_hid], F32, tag="r2")
    nc.vector.tensor_tensor(
        out=r2_sb[:], in0=r_sb[:], in1=b1_sb[:], op=mybir.AluOpType.add
    )
    r3_sb = wpool.tile([WINDOW, d_hid], F32, tag="r3")
    nc.scalar.activation(out=r3_sb[:], in_=r2_sb[:], func=Relu)
    # transpose -> [hid, dst]
    rT_ps = psW.tile([d_hid, WINDOW], F32, tag="wps", name="rT_ps")
    nc.tensor.transpose(out=rT_ps[:], in_=r3_sb[:], identity=id_sb[:])
    rT_sb = wpool.tile([d_hid, WINDOW], F32, tag="rTs")
    nc.scalar.activation(out=rT_sb[:], in_=rT_ps[:], func=Copy)
    # h2 [dst, n_cls] = rT.T @ W2; scale by dinv[dst]; replicate REP x
    h2_ps = psW.tile([WINDOW, n_cls], F32, tag="wps", name="h2_ps")
    nc.tensor.matmul(out=h2_ps[:], lhsT=rT_sb[:], rhs=w2_sb[:], start=True, stop=True)
    h2_sb = wpool.tile([WINDOW, d_rep], BF16, tag="h2s")
    nc.vector.tensor_scalar(
        out=h2_sb[:].rearrange("p (r c) -> p r c", c=n_cls),
        in0=h2_ps[:]
        .rearrange("p (one c) -> p one c", one=1)
        .to_broadcast([WINDOW, REP, n_cls]),
        scalar1=dinvw_sb[:, w : w + 1],
        scalar2=None,
        op0=mybir.AluOpType.mult,
    )
    nrows = min(WINDOW, n_local - w * WINDOW)
    nc.sync.dma_start(
        out=h2loc[w * WINDOW : w * WINDOW + nrows, :], in_=h2_sb[:nrows, :]
    )


# --------------------------------------------------------------------------
# Entry point
# --------------------------------------------------------------------------
def _make_inputs(x, W1, b1, W2, b2, pp, dt_np):
    N, d_in = x.shape
    W1 = np.asarray(W1, np.float32)
    b1 = np.asarray(b1, np.float32)
    W2 = np.asarray(W2, np.float32)
    b2 = np.asarray(b2, np.float32)
    d_hid = W1.shape[1]
    n_cls = W2.shape[1]
    xtab = np.concatenate(
        [
            np.zeros((1, d_in), np.float32),
            x * pp["dinv"][:, None],
            np.zeros((1, d_in), np.float32),
        ]
    ).astype(dt_np)
    iota_arr = np.broadcast_to(
        np.tile(np.arange(WINDOW, dtype=np.float32), SBATCH),
        (CHUNK, SBATCH * WINDOW),
    ).copy()
    shared = {
        "xtab": xtab,
        "w1": W1,
        "w2": W2,
        "b1bc": np.broadcast_to(b1, (WINDOW, d_hid)).astype(np.float32).copy(),
        "b2bc": np.broadcast_to(b2, (WINDOW, n_cls)).astype(np.float32).copy(),
        "iota": iota_arr,
        "ident": np.eye(WINDOW, dtype=np.float32),
    }
    in_maps = []
    for pc in pp["per_core"]:
        m = dict(shared)
        m["idx16"] = pc["idx16"]
        m["dstrel"] = pc["dstrel"]
        m["dinvw"] = pc["dinvw"]
        in_maps.append(m)
    return in_maps


def _run(x, edge_index, W1, b1, W2, b2, n_cores, trace=False):
    x = np.asarray(x, dtype=np.float32)
    N, d_in = x.shape
    d_hid = np.asarray(W1).shape[1]
    n_cls = np.asarray(W2).shape[1]
    assert d_in == 128 and d_hid == 128

    pp = _preprocess(x, edge_index, n_cores)
    dt_gat = BF16 if GATHER_BF16 else F32
    np_gat = np.dtype("bfloat16") if GATHER_BF16 else np.dtype("float32")

    nc = bacc.Bacc("TRN2", target_bir_lowering=False, debug=False,
                   num_swdge_queues=N_QUEUES,
                   dynamic_dma_scratch_size=65536)
    _build(
        nc,
        N=N,
        n_local=pp["n_local"],
        d_in=d_in,
        d_hid=d_hid,
        n_cls=n_cls,
        pp=pp,
        n_cores=n_cores,
        dt_gat=dt_gat,
    )

    import ml_dtypes  # noqa

    in_maps = _make_inputs(x, W1, b1, W2, b2, pp, np_gat)
    res = run_bass_kernel_spmd(nc, in_maps, list(range(n_cores)), trace=trace)
    outs = [res.results[c]["out"] for c in range(n_cores)]
    full = np.concatenate(outs, axis=0)[:N]
    return full.astype(np.float32), res


def kernel(x, edge_index, W1, b1, W2, b2):
    out, _ = _run(x, edge_index, W1, b1, W2, b2, N_CORES)
    return out



# revision 13
# speedup vs baseline: 1.2236x; 1.1957x over previous
"""GCN 2-layer (PyG GCNConv x2 + ReLU) Bass kernel for Trainium2, 8-core SPMD.

V2 strategy:
  Phase A (layer 1): dma_gather (queues 0-3 round-robin; q1-3 DGE runs async
    on idle Q7 core pairs) of prescaled-x bf16 rows -> G [128e, d_in]; one-hot
    S on DVE; PE accumulates G.T @ S into PSUM [d_in, 128dst] per window.
    Self-loops removed from the edge list; their dinv^2*x contribution is a
    host-precomputed transposed slab added at LOW-section eviction.
    Per window epilogue: @W1, dinv[dst] scale, +b1, ReLU, transpose, @W2 ->
    raw h2 [dst, 2] packed to one bf16-pair word per node.
  Exchange: AllGather of [128, w_cnt] word shards (25KB/core vs 1.6MB in V1).
  Phase B (layer 2): h2 word table broadcast into SBUF (HIGH then LOW
    section, <=32768 words each for int16 ap_gather addressing); ONE big
    split-stream ap_gather per section (8 Q7 cores, independent 64-edge
    chunks); per 8-chunk group one PE transpose turns replicated value rows
    into [2j+b, chunk] layout; R [128pv, 2] = transposed col x normmask
    routes class values; flipped matmul R.T @ S accumulates [2, dst] per
    window; per-window fixup adds self term + b2 and writes out rows.
"""

import numpy as np

import concourse.bass as bass
import concourse.mybir as mybir
import concourse.tile as tile
from concourse import bacc
from concourse.bass_utils import run_bass_kernel_spmd

F32 = mybir.dt.float32
BF16 = mybir.dt.bfloat16
I16 = mybir.dt.int16

N_CORES = 8
WINDOW = 128  # dst nodes per PSUM accumulation window
CHUNK = 128  # edges per phase-A matmul chunk
BCHUNK = 64  # edges per phase-B chunk (2 bf16 rows per edge after transpose)
GSZ = 8  # chunks per dma_gather instruction
SBATCH = 8  # chunks per S-build DVE op
HALF = 32768  # int16 index range
N_QUEUES = 4


# --------------------------------------------------------------------------
# Host preprocessing
# --------------------------------------------------------------------------
def _preprocess(x, edge_index, n_cores):
    N = x.shape[0]
    e_src = np.asarray(edge_index[0], dtype=np.int64)
    e_dst = np.asarray(edge_index[1], dtype=np.int64)
    deg = (np.bincount(e_dst, minlength=N) + 1).astype(np.float64)
    dinv = (1.0 / np.sqrt(deg)).astype(np.float32)

    assert N % n_cores == 0
    n_local = N // n_cores
    w_cnt = (n_local + WINDOW - 1) // WINDOW

    order = np.argsort(e_dst, kind="stable")
    s_src = e_src[order]
    s_dst = e_dst[order]

    # ---------------- phase A (row split by src+1 vs HALF) ----------------
    HB = N + 2 - HALF
    pad_high = N + 1 - HB  # zero row N+1 relative to HB

    partsA = {}
    cntA = np.zeros((2, n_cores, w_cnt), dtype=np.int64)
    # ---------------- phase B (pos split) ----------------
    stride_c = w_cnt * WINDOW
    pos_of = np.empty(N, dtype=np.int64)
    v = np.arange(N, dtype=np.int64)
    r = v % n_local
    pos_of = (v // n_local) * stride_c + (r % WINDOW) * w_cnt + r // WINDOW
    NPOS = n_cores * stride_c
    partsB = {}
    cntB = np.zeros((2, n_cores, w_cnt), dtype=np.int64)

    for c in range(n_cores):
        base = c * n_local
        for w in range(w_cnt):
            wlo = base + w * WINDOW
            whi = min(base + (w + 1) * WINDOW, base + n_local)
            lo_i = np.searchsorted(s_dst, wlo, side="left")
            hi_i = np.searchsorted(s_dst, whi, side="left")
            srcs = s_src[lo_i:hi_i]
            rel = (s_dst[lo_i:hi_i] - wlo).astype(np.float32)
            rows = srcs + 1
            is_lo = rows < HALF
            partsA[(c, w, 0)] = (rows[is_lo], rel[is_lo])
            partsA[(c, w, 1)] = (rows[~is_lo] - HB, rel[~is_lo])
            cntA[0, c, w] = is_lo.sum()
            cntA[1, c, w] = (~is_lo).sum()
            spos = pos_of[srcs]
            nrm = (dinv[srcs] * dinv[s_dst[lo_i:hi_i]]).astype(np.float32)
            b_hi = spos >= HALF  # section 0 = HIGH (processed first)
            partsB[(c, w, 0)] = (spos[b_hi] - HALF, rel[b_hi], nrm[b_hi])
            partsB[(c, w, 1)] = (spos[~b_hi], rel[~b_hi], nrm[~b_hi])
            cntB[0, c, w] = b_hi.sum()
            cntB[1, c, w] = (~b_hi).sum()

    # ---- phase A chunk scaffold (uniform across cores) ----
    kwA = [np.maximum(1, np.ceil(cntA[s] / CHUNK).astype(np.int64).max(axis=0))
           for s in range(2)]
    TA_lo, TA_hi = int(kwA[0].sum()), int(kwA[1].sum())
    TA = TA_lo + TA_hi
    chunkA = []  # (w, first, last, sec)
    for sec in range(2):
        for w in range(w_cnt):
            for k in range(kwA[sec][w]):
                chunkA.append((w, k == 0, k == kwA[sec][w] - 1, sec))

    # ---- phase B chunk scaffold ----
    kwB = [np.maximum(1, np.ceil(cntB[s] / BCHUNK).astype(np.int64).max(axis=0))
           for s in range(2)]
    for s in range(2):
        tot = int(kwB[s].sum())
        kwB[s][w_cnt - 1] += (-tot) % 8  # whole groups of 8
    TB = [int(kwB[s].sum()) for s in range(2)]
    chunkB = [[], []]  # per sec: (w, first, last)
    for sec in range(2):
        for w in range(w_cnt):
            for k in range(kwB[sec][w]):
                chunkB[sec].append((w, k == 0, k == kwB[sec][w] - 1))

    per_core = []
    for c in range(n_cores):
        idxA = np.zeros(TA * CHUNK, dtype=np.int32)
        dstrelA = np.zeros((CHUNK, TA), dtype=np.float32)
        t = 0
        for sec, padrow in ((0, 0), (1, pad_high)):
            for w in range(w_cnt):
                rows, rel = partsA[(c, w, sec)]
                n_e = len(rows)
                n_slots = int(kwA[sec][w]) * CHUNK
                buf = np.full(n_slots, padrow, dtype=np.int32)
                buf[:n_e] = rows
                idxA[t * CHUNK:t * CHUNK + n_slots] = buf
                rbuf = np.zeros(n_slots, dtype=np.float32)
                rbuf[:n_e] = rel
                dstrelA[:, t:t + int(kwA[sec][w])] = rbuf.reshape(-1, CHUNK).T
                t += int(kwA[sec][w])
        assert t == TA
        idx16 = idxA.astype(np.int16).reshape(-1, 16).T
        idx16 = np.tile(idx16, (8, 1))

        apidx = []
        rel2 = np.full((2 * BCHUNK, TB[0] + TB[1]), -1.0, dtype=np.float32)
        normmask = np.zeros((2 * BCHUNK, 2 * (TB[0] + TB[1])), dtype=np.float32)
        tglob = 0
        for sec in range(2):
            n_grp = TB[sec] // 8
            streams = np.zeros((8, n_grp * BCHUNK), dtype=np.int16)
            t = 0
            for w in range(w_cnt):
                pos_list, rel_list, nrm_list = partsB[(c, w, sec)]
                p = 0
                for k in range(int(kwB[sec][w])):
                    n_e = max(min(BCHUNK, len(pos_list) - p), 0)
                    g, q = t // 8, t % 8
                    if n_e > 0:
                        streams[q, g * BCHUNK:g * BCHUNK + n_e] = pos_list[
                            p:p + n_e].astype(np.int16)
                        j = np.arange(n_e)
                        rel2[2 * j, tglob] = rel_list[p:p + n_e]
                        rel2[2 * j + 1, tglob] = rel_list[p:p + n_e]
                        normmask[2 * j, 2 * tglob] = nrm_list[p:p + n_e]
                        normmask[2 * j + 1, 2 * tglob + 1] = nrm_list[p:p + n_e]
                    p += n_e
                    t += 1
                    tglob += 1
            assert t == TB[sec]
            num_idxs = n_grp * BCHUNK
            ap = np.zeros((128, num_idxs // 16), dtype=np.int16)
            for q in range(8):
                ap[16 * q:16 * (q + 1), :] = streams[q].reshape(-1, 16).T
            apidx.append(ap)

        # self term enters aggT pre-W1; the epilogue multiplies by dinv[dst],
        # so prescale by a single dinv power to end at dinv^2 * xW1.
        xselfT = (np.asarray(x[c * n_local:(c + 1) * n_local], np.float32)
                  * dinv[c * n_local:(c + 1) * n_local][:, None]).T.copy()

        dinvw = np.zeros((WINDOW, w_cnt), dtype=np.float32)
        dinv2w = np.zeros((WINDOW, w_cnt), dtype=np.float32)
        dv = dinv[c * n_local:(c + 1) * n_local]
        for w in range(w_cnt):
            seg = dv[w * WINDOW:(w + 1) * WINDOW]
            dinvw[:len(seg), w] = seg
            dinv2w[:len(seg), w] = seg ** 2
        per_core.append({
            "idx16": idx16, "dstrel": dstrelA, "dinvw": dinvw,
            "dinv2w": dinv2w, "xselfT": xselfT,
            "apidxH": apidx[0], "apidxL": apidx[1],
            "rel2": rel2, "normmask": normmask,
        })

    return {
        "n_local": n_local, "w_cnt": w_cnt, "TA": TA, "TA_lo": TA_lo,
        "HB": HB, "chunkA": chunkA, "chunkB": chunkB, "TB": TB,
        "NPOS": NPOS, "dinv": dinv, "per_core": per_core,
    }


# --------------------------------------------------------------------------
# Device kernel builder
# --------------------------------------------------------------------------
def _build(nc, *, N, n_local, d_in, d_hid, n_cls, pp, n_cores):
    Relu = mybir.ActivationFunctionType.Relu
    Copy = mybir.ActivationFunctionType.Copy
    TA, TA_lo = pp["TA"], pp["TA_lo"]
    w_cnt, HB = pp["w_cnt"], pp["HB"]
    chunkA, chunkB, TB = pp["chunkA"], pp["chunkB"], pp["TB"]
    NPOS = pp["NPOS"]
    NHI = NPOS - HALF
    nIH = TB[0] // 8 * BCHUNK
    nIL = TB[1] // 8 * BCHUNK

    xtab = nc.dram_tensor("xtab", [N + 2, d_in], BF16, kind="ExternalInput")
    w1 = nc.dram_tensor("w1", [d_in, d_hid], F32, kind="ExternalInput")
    w2 = nc.dram_tensor("w2", [d_hid, n_cls], F32, kind="ExternalInput")
    b1bc = nc.dram_tensor("b1bc", [WINDOW, d_hid], F32, kind="ExternalInput")
    b2bc = nc.dram_tensor("b2bc", [WINDOW, n_cls], F32, kind="ExternalInput")
    iota = nc.dram_tensor("iota", [CHUNK, SBATCH * WINDOW], F32,
                          kind="ExternalInput")
    ident = nc.dram_tensor("ident", [WINDOW, WINDOW], F32, kind="ExternalInput")
    identb = nc.dram_tensor("identb", [WINDOW, WINDOW], BF16,
                            kind="ExternalInput")
    idx_t = nc.dram_tensor("idx16", [CHUNK, TA * 8], I16, kind="ExternalInput")
    dstrel_t = nc.dram_tensor("dstrel", [CHUNK, TA], F32, kind="ExternalInput")
    dinvw_t = nc.dram_tensor("dinvw", [WINDOW, w_cnt], F32, kind="ExternalInput")
    dinv2w_t = nc.dram_tensor("dinv2w", [WINDOW, w_cnt], F32,
                              kind="ExternalInput")
    xselfT_t = nc.dram_tensor("xselfT", [d_in, n_local], F32,
                              kind="ExternalInput")
    apidxH_t = nc.dram_tensor("apidxH", [128, nIH // 16], I16,
                              kind="ExternalInput")
    apidxL_t = nc.dram_tensor("apidxL", [128, nIL // 16], I16,
                              kind="ExternalInput")
    rel2_t = nc.dram_tensor("rel2", [2 * BCHUNK, TB[0] + TB[1]], F32,
                            kind="ExternalInput")
    nmask_t = nc.dram_tensor("nmask", [2 * BCHUNK, 2 * (TB[0] + TB[1])], F32,
                             kind="ExternalInput")
    out_t = nc.dram_tensor("out", [n_local, n_cls], F32, kind="ExternalOutput")

    h2loc = nc.dram_tensor("h2loc", [128, w_cnt], F32)
    h2all = nc.dram_tensor("h2all", [n_cores * 128, w_cnt], F32,
                           addr_space="Shared")

    groupsA = []
    for sec, tlo, thi in ((0, 0, TA_lo), (1, TA_lo, TA)):
        t0 = tlo
        while t0 < thi:
            n = min(GSZ, thi - t0)
            groupsA.append((sec, t0, n))
            t0 += n

    def tab_view(tab):
        return [tab[0:HALF, :], tab[HB:HB + HALF, :]]

    with tile.TileContext(nc) as tc:
        with tc.tile_pool(name="persist", bufs=1) as ppool:
            b2_sb = ppool.tile([WINDOW, n_cls], F32, tag="b2")
            nc.sync.dma_start(out=b2_sb[:], in_=b2bc[:])
            idb_sb = ppool.tile([WINDOW, WINDOW], BF16, tag="idb")
            nc.sync.dma_start(out=idb_sb[:], in_=identb[:])
            iota_sb = ppool.tile([CHUNK, SBATCH * WINDOW], F32, tag="iota")
            nc.sync.dma_start(out=iota_sb[:], in_=iota[:])
            dinv2w_sb = ppool.tile([WINDOW, w_cnt], F32, tag="dinv2w")
            nc.sync.dma_start(out=dinv2w_sb[:], in_=dinv2w_t[:])
            apidxH_sb = ppool.tile([128, nIH // 16], I16, tag="apidxH")
            nc.sync.dma_start(out=apidxH_sb[:], in_=apidxH_t[:])
            apidxL_sb = ppool.tile([128, nIL // 16], I16, tag="apidxL")
            nc.sync.dma_start(out=apidxL_sb[:], in_=apidxL_t[:])
            rel2_sb = ppool.tile([2 * BCHUNK, TB[0] + TB[1]], F32, tag="rel2")
            nc.sync.dma_start(out=rel2_sb[:], in_=rel2_t[:])
            nmask_sb = ppool.tile([2 * BCHUNK, 2 * (TB[0] + TB[1])], F32,
                                  tag="nmask")
            nc.sync.dma_start(out=nmask_sb[:], in_=nmask_t[:])
            h2loc_sb = ppool.tile([128, w_cnt], F32, tag="h2loc")
            o2 = ppool.tile([WINDOW, w_cnt * n_cls], F32, tag="o2")

            # ========================= PHASE A =========================
            with (
                tc.tile_pool(name="aconst", bufs=1) as acpool,
                tc.tile_pool(name="gbuf", bufs=10) as gpool,
                tc.tile_pool(name="sbat", bufs=3) as spool,
                tc.tile_pool(name="wtmp", bufs=3) as wpool,
                tc.tile_pool(name="psA", bufs=3, space="PSUM") as psA,
                tc.tile_pool(name="psW", bufs=3, space="PSUM") as psW,
            ):
                w1_sb = acpool.tile([d_in, d_hid], F32, tag="w1")
                nc.sync.dma_start(out=w1_sb[:], in_=w1[:])
                w2_sb = acpool.tile([d_hid, n_cls], F32, tag="w2")
                nc.sync.dma_start(out=w2_sb[:], in_=w2[:])
                b1_sb = acpool.tile([WINDOW, d_hid], F32, tag="b1")
                nc.sync.dma_start(out=b1_sb[:], in_=b1bc[:])
                id_sb = acpool.tile([WINDOW, WINDOW], F32, tag="ident")
                nc.sync.dma_start(out=id_sb[:], in_=ident[:])
                idx_sb = acpool.tile([CHUNK, TA * 8], I16, tag="idx")
                nc.sync.dma_start(out=idx_sb[:], in_=idx_t[:])
                dstrel_sb = acpool.tile([CHUNK, TA], F32, tag="dstrel")
                nc.sync.dma_start(out=dstrel_sb[:], in_=dstrel_t[:])
                dinvw_sb = acpool.tile([WINDOW, w_cnt], F32, tag="dinvw")
                nc.sync.dma_start(out=dinvw_sb[:], in_=dinvw_t[:])
                xselfT_sb = acpool.tile([d_in, n_local], F32, tag="xselfT")
                nc.sync.dma_start(out=xselfT_sb[:], in_=xselfT_t[:])
                aggT_sb = acpool.tile([d_in, w_cnt * WINDOW], F32, tag="aggT")

                def build_sA(t0, n):
                    s_tile = spool.tile([CHUNK, SBATCH * WINDOW], BF16,
                                        tag="s", name="sA")
                    rel_b = (dstrel_sb[:, t0:t0 + n]
                             .rearrange("p (b one) -> p b one", one=1)
                             .to_broadcast([CHUNK, n, WINDOW]))
                    io_v = iota_sb[:, :n * WINDOW].rearrange(
                        "p (b j) -> p b j", j=WINDOW)
                    s_v = s_tile[:, :n * WINDOW].rearrange(
                        "p (b j) -> p b j", j=WINDOW)
                    nc.vector.tensor_tensor(out=s_v, in0=io_v, in1=rel_b,
                                            op=mybir.AluOpType.is_equal)
                    return s_tile

                psum_of_win = {}
                for gi, (sec, t0, n) in enumerate(groupsA):
                    gb = gpool.tile([CHUNK, GSZ, d_in], BF16, tag="g",
                                    name="gb")
                    nc.gpsimd.dma_gather(
                        gb[:, :n, :], tab_view(xtab)[sec],
                        idx_sb[:, t0 * 8:(t0 + n) * 8],
                        n * CHUNK, n * CHUNK, d_in, single_packet=True,
                        queue_num=gi % N_QUEUES,
                    )
                    for bt0 in range(t0, t0 + n, SBATCH):
                        bn = min(SBATCH, t0 + n - bt0)
                        s_tile = build_sA(bt0, bn)
                        for t in range(bt0, bt0 + bn):
                            j = t - bt0
                            w, first, last, _sec = chunkA[t]
                            if first:
                                psum_of_win[w] = psA.tile(
                                    [d_in, WINDOW], F32, tag="agg",
                                    name="aggps")
                            nc.tensor.matmul(
                                out=psum_of_win[w][:],
                                lhsT=gb[:, t - t0, :],
                                rhs=s_tile[:, j * WINDOW:(j + 1) * WINDOW],
                                start=first, stop=last,
                            )
                            if not last:
                                continue
                            ps = psum_of_win.pop(w)
                            wsl = aggT_sb[:, w * WINDOW:(w + 1) * WINDOW]
                            nw = min(WINDOW, n_local - w * WINDOW)
                            if _sec == 0:
                                nc.vector.tensor_tensor(
                                    out=wsl[:, :nw], in0=ps[:, :nw],
                                    in1=xselfT_sb[:, w * WINDOW:w * WINDOW + nw],
                                    op=mybir.AluOpType.add)
                                if nw < WINDOW:
                                    nc.scalar.activation(
                                        out=wsl[:, nw:], in_=ps[:, nw:],
                                        func=Copy)
                            else:
                                nc.vector.tensor_tensor(
                                    out=wsl, in0=ps[:], in1=wsl,
                                    op=mybir.AluOpType.add)
                                h1_ps = psW.tile([WINDOW, d_hid], F32,
                                                 tag="wps", name="h1ps")
                                nc.tensor.matmul(out=h1_ps[:], lhsT=wsl,
                                                 rhs=w1_sb[:], start=True,
                                                 stop=True)
                                r_sb = wpool.tile([WINDOW, d_hid], F32, tag="r")
                                nc.vector.tensor_scalar(
                                    out=r_sb[:], in0=h1_ps[:],
                                    scalar1=dinvw_sb[:, w:w + 1], scalar2=None,
                                    op0=mybir.AluOpType.mult)
                                r2_sb = wpool.tile([WINDOW, d_hid], F32,
                                                   tag="r2")
                                nc.vector.tensor_tensor(
                                    out=r2_sb[:], in0=r_sb[:], in1=b1_sb[:],
                                    op=mybir.AluOpType.add)
                                r3_sb = wpool.tile([WINDOW, d_hid], F32,
                                                   tag="r3")
                                nc.scalar.activation(out=r3_sb[:],
                                                     in_=r2_sb[:], func=Relu)
                                rT_ps = psW.tile([d_hid, WINDOW], F32,
                                                 tag="wps", name="rTps")
                                nc.tensor.transpose(out=rT_ps[:], in_=r3_sb[:],
                                                    identity=id_sb[:])
                                rT_sb = wpool.tile([d_hid, WINDOW], F32,
                                                   tag="rTs")
                                nc.scalar.activation(out=rT_sb[:],
                                                     in_=rT_ps[:], func=Copy)
                                h2_ps = psW.tile([WINDOW, n_cls], F32,
                                                 tag="wps", name="h2ps")
                                nc.tensor.matmul(out=h2_ps[:], lhsT=rT_sb[:],
                                                 rhs=w2_sb[:], start=True,
                                                 stop=True)
                                nc.vector.tensor_scalar(
                                    out=h2loc_sb[:, w:w + 1].bitcast(BF16),
                                    in0=h2_ps[:], scalar1=1.0, scalar2=None,
                                    op0=mybir.AluOpType.mult)

            # ======================= h2 exchange =======================
            nc.sync.dma_start(out=h2loc[:], in_=h2loc_sb[:])
            if n_cores > 1:
                nc.gpsimd.collective_compute(
                    "AllGather", mybir.AluOpType.bypass,
                    replica_groups=[list(range(n_cores))],
                    ins=[h2loc[:]], outs=[h2all[:]],
                )
            else:
                nc.sync.dma_start(out=h2all[0:128, :], in_=h2loc[:])

            # ========================= PHASE B =========================
            h2flat = h2all.rearrange("a b -> (a b)")
            with (
                tc.tile_pool(name="tab", bufs=1) as tabpool,
                tc.tile_pool(name="ogH", bufs=1) as ogHpool,
                tc.tile_pool(name="ogL", bufs=1) as ogLpool,
                tc.tile_pool(name="sbat2", bufs=3) as s2pool,
                tc.tile_pool(name="rbuf", bufs=4) as rpool,
                tc.tile_pool(name="wtmp2", bufs=4) as w2pool,
                tc.tile_pool(name="ps2", bufs=3, space="PSUM") as ps2,
                tc.tile_pool(name="psT", bufs=3, space="PSUM") as psTp,
                tc.tile_pool(name="psF", bufs=2, space="PSUM") as psFp,
            ):
                def run_section(sec, tabw, flat_lo, apidx_sb, nI, tbase,
                                ogpool):
                    tabt = tabpool.tile([128, HALF], F32, tag="tab",
                                        name=f"tab{sec}")
                    src = (h2flat[flat_lo:flat_lo + tabw]
                           .rearrange("(one w) -> one w", one=1)
                           .to_broadcast([128, tabw]))
                    nc.sync.dma_start(out=tabt[:, :tabw], in_=src)
                    og = ogpool.tile([128, nI], F32, tag="og", name=f"og{sec}")
                    nc.gpsimd.ap_gather(
                        og[:].rearrange("p (i one) -> p i one", one=1),
                        tabt[:, :tabw].rearrange("p (e one) -> p e one", one=1),
                        apidx_sb[:], 128, tabw, 1, nI,
                    )
                    psum_of_win = {}
                    for g in range(TB[sec] // 8):
                        ogv = og[:, g * BCHUNK:(g + 1) * BCHUNK].bitcast(BF16)
                        psT_t = psTp.tile([128, 128], BF16, tag="pst",
                                          name="psTt")
                        nc.tensor.transpose(out=psT_t[:], in_=ogv,
                                            identity=idb_sb[:])
                        r8 = rpool.tile([128, 8, n_cls], BF16, tag="r8")
                        in0 = (psT_t[:].rearrange("p (k s) -> p k s", s=16)
                               [:, :, 0:1].to_broadcast([128, 8, n_cls]))
                        in1 = nmask_sb[:, 2 * (tbase + 8 * g):
                                       2 * (tbase + 8 * g) + 16].rearrange(
                            "p (k two) -> p k two", two=n_cls)
                        nc.vector.tensor_tensor(out=r8[:], in0=in0, in1=in1,
                                                op=mybir.AluOpType.mult)
                        s_tile = s2pool.tile([2 * BCHUNK, 8 * WINDOW], BF16,
                                             tag="s2", name="sB")
                        rel_b = (rel2_sb[:, tbase + 8 * g:tbase + 8 * g + 8]
                                 .rearrange("p (b one) -> p b one", one=1)
                                 .to_broadcast([2 * BCHUNK, 8, WINDOW]))
                        io_v = iota_sb[:, :8 * WINDOW].rearrange(
                            "p (b j) -> p b j", j=WINDOW)
                        s_v = s_tile[:].rearrange("p (b j) -> p b j", j=WINDOW)
                        nc.vector.tensor_tensor(out=s_v, in0=io_v, in1=rel_b,
                                                op=mybir.AluOpType.is_equal)
                        for k in range(8):
                            t = 8 * g + k
                            w, first, last = chunkB[sec][t]
                            if first:
                                psum_of_win[w] = ps2.tile(
                                    [n_cls, WINDOW], F32, tag="o", name="ops")
                            nc.tensor.matmul(
                                out=psum_of_win[w][:],
                                lhsT=r8[:, k, :],
                                rhs=s_tile[:, k * WINDOW:(k + 1) * WINDOW],
                                start=first, stop=last,
                            )
                            if not last:
                                continue
                            ps = psum_of_win.pop(w)
                            e_sb = w2pool.tile([n_cls, WINDOW], BF16, tag="ev")
                            nc.vector.tensor_scalar(
                                out=e_sb[:], in0=ps[:], scalar1=1.0,
                                scalar2=None, op0=mybir.AluOpType.mult)
                            f_ps = psFp.tile([WINDOW, n_cls], BF16, tag="f",
                                             name="fps")
                            nc.tensor.transpose(
                                out=f_ps[:], in_=e_sb[:],
                                identity=idb_sb[:n_cls, :n_cls])
                            osl = o2[:, w * n_cls:(w + 1) * n_cls]
                            if sec == 0:
                                nc.scalar.activation(out=osl, in_=f_ps[:],
                                                     func=Copy)
                            else:
                                t1 = w2pool.tile([WINDOW, n_cls], F32,
                                                 tag="t1")
                                nc.vector.tensor_tensor(
                                    out=t1[:], in0=f_ps[:], in1=osl,
                                    op=mybir.AluOpType.add)
                                t2 = w2pool.tile([WINDOW, n_cls], F32,
                                                 tag="t2")
                                nc.vector.tensor_scalar(
                                    out=t2[:],
                                    in0=h2loc_sb[:, w:w + 1].bitcast(BF16),
                                    scalar1=dinv2w_sb[:, w:w + 1],
                                    scalar2=None, op0=mybir.AluOpType.mult)
                                t3 = w2pool.tile([WINDOW, n_cls], F32,
                                                 tag="t3")
                                nc.vector.tensor_tensor(
                                    out=t3[:], in0=t2[:], in1=b2_sb[:],
                                    op=mybir.AluOpType.add)
                                t4 = w2pool.tile([WINDOW, n_cls], F32,
                                                 tag="t4")
                                nc.vector.tensor_tensor(
                                    out=t4[:], in0=t1[:], in1=t3[:],
                                    op=mybir.AluOpType.add)
                                nw = min(WINDOW, n_local - w * WINDOW)
                                nc.sync.dma_start(
                                    out=out_t[w * WINDOW:w * WINDOW + nw, :],
                                    in_=t4[:nw, :])

                run_section(0, NHI, HALF, apidxH_sb, nIH, 0, ogHpool)
                run_section(1, HALF, 0, apidxL_sb, nIL, TB[0], ogLpool)

    nc.compile()
    return nc


# --------------------------------------------------------------------------
# Entry point
# --------------------------------------------------------------------------
def _make_inputs(x, W1, b1, W2, b2, pp):
    import ml_dtypes  # noqa
    N, d_in = x.shape
    W1 = np.asarray(W1, np.float32)
    b1 = np.asarray(b1, np.float32)
    W2 = np.asarray(W2, np.float32)
    b2 = np.asarray(b2, np.float32)
    d_hid = W1.shape[1]
    n_cls = W2.shape[1]
    xtab = np.concatenate([
        np.zeros((1, d_in), np.float32),
        x * pp["dinv"][:, None],
        np.zeros((1, d_in), np.float32),
    ]).astype(np.dtype("bfloat16"))
    iota_arr = np.broadcast_to(
        np.tile(np.arange(WINDOW, dtype=np.float32), SBATCH),
        (CHUNK, SBATCH * WINDOW)).copy()
    shared = {
        "xtab": xtab,
        "w1": W1,
        "w2": W2,
        "b1bc": np.broadcast_to(b1, (WINDOW, d_hid)).astype(np.float32).copy(),
        "b2bc": np.broadcast_to(b2, (WINDOW, n_cls)).astype(np.float32).copy(),
        "iota": iota_arr,
        "ident": np.eye(WINDOW, dtype=np.float32),
        "identb": np.eye(WINDOW, dtype=np.float32).astype(np.dtype("bfloat16")),
    }
    in_maps = []
    for pc in pp["per_core"]:
        m = dict(shared)
        m["idx16"] = pc["idx16"]
        m["dstrel"] = pc["dstrel"]
        m["dinvw"] = pc["dinvw"]
        m["dinv2w"] = pc["dinv2w"]
        m["xselfT"] = pc["xselfT"]
        m["apidxH"] = pc["apidxH"]
        m["apidxL"] = pc["apidxL"]
        m["rel2"] = pc["rel2"]
        m["nmask"] = pc["normmask"]
        in_maps.append(m)
    return in_maps


def _run(x, edge_index, W1, b1, W2, b2, n_cores, trace=False):
    x = np.asarray(x, dtype=np.float32)
    N, d_in = x.shape
    d_hid = np.asarray(W1).shape[1]
    n_cls = np.asarray(W2).shape[1]
    assert d_in == 128 and d_hid == 128

    pp = _preprocess(x, edge_index, n_cores)

    nc = bacc.Bacc("TRN2", target_bir_lowering=False, debug=False,
                   num_swdge_queues=N_QUEUES,
                   dynamic_dma_scratch_size=16384)
    _build(nc, N=N, n_local=pp["n_local"], d_in=d_in, d_hid=d_hid,
           n_cls=n_cls, pp=pp, n_cores=n_cores)

    in_maps = _make_inputs(x, W1, b1, W2, b2, pp)
    res = run_bass_kernel_spmd(nc, in_maps, list(range(n_cores)), trace=trace)
    outs = [res.results[c]["out"] for c in range(n_cores)]
    full = np.concatenate(outs, axis=0)[:N]
    return full.astype(np.float32), res


def kernel(x, edge_index, W1, b1, W2, b2):
    out, _ = _run(x, edge_index, W1, b1, W2, b2, N_CORES)
    return out
